# revision 11
# baseline (speedup 1.0000x reference)
"""GridSpatialIntegral Trainium2 kernel.

Reference computes, for input [B=32, 2, 512, 512] f32:
  out[:, 0] = cumsum(input[:, 0], axis=-1)   (along width, contiguous axis)
  out[:, 1] = cumsum(input[:, 1], axis=-2)   (along height)

Strategy (data-parallel over batch, 4 images/core on 8 cores):
  - channel 0: rows on partitions, native DVE prefix scan
    (tensor_tensor_scan, op0=add/op1=bypass) along the free axis.
  - channel 1: height lives on partitions in 4 chunks of 128
    (h = j*128 + p). Chunk j's full 512-prefix is computed entirely in
    PE with one PSUM accumulation group per chunk:
        psum_j = U@x_0 + ... + U@x_{j-1} + T@x_j
    where T is upper-triangular ones (within-chunk inclusive prefix)
    and U is all-ones (broadcasts each earlier chunk's column totals to
    every partition). No inter-chunk carry chain, no partition
    broadcast, no compute-dependent DMAs. Matmuls run as float32r
    (1 cycle/row at N=512 vs 4 for fp32; a 0/1 weight matrix is exact).
    T/U are generated on-chip (memset + affine_select), which both
    avoids two const DMAs and satisfies the BIR verifier's
    "fp32r operands must come from a rounding compute op" rule; the
    image data is rounded to fp32r by DVE tensor_copy.
  - PSUM->SBUF moves on the Activation engine; DVE runs the rounding
    copies and the channel-0 scans.

  DMA queue discipline (the kernel is DMA-roofline-bound at ~16.8 MB
  per core): one 2 MiB load per image on SP's HWDGE ring -- nothing
  else ever sits on that ring, so the four loads stream back-to-back;
  one 2 MiB store per image on gpsimd's SWDGE ring -- a store waiting
  on compute can never head-of-line block a load or a PSUM drain.
"""

import numpy as np
from contextlib import ExitStack

B, C, H, W = 32, 2, 512, 512
NCORES = 8
BLOC = B // NCORES  # images per core
P = 128             # SBUF partitions
NCH = H // P        # 128-row chunks per image

_compiled = None


def _build():
    import concourse.bacc as bacc
    import concourse.tile as tile
    from concourse import mybir

    nc = bacc.Bacc(
        "TRN2",
        target_bir_lowering=False,
        debug=False,
        enable_asserts=False,
        num_devices=1,
    )
    x = nc.dram_tensor("x", (BLOC, C, H, W), mybir.dt.float32, kind="ExternalInput").ap()
    y = nc.dram_tensor("y", (BLOC, C, H, W), mybir.dt.float32, kind="ExternalOutput").ap()

    add = mybir.AluOpType.add
    bypass = mybir.AluOpType.bypass
    f32r = mybir.dt.float32r

    with tile.TileContext(nc) as tc, ExitStack() as ctx:
        # out_pool is entered FIRST: ExitStack closes pools in LIFO order,
        # and each close emits a release sem-wait on SP's sequencer. The
        # out tiles' last reader is the final store DMA, so its wait is the
        # only late one -- making it the last emitted keeps the other pool
        # waits off the post-store critical path.
        out_pool = ctx.enter_context(tc.tile_pool(name="out", bufs=BLOC))
        const_pool = ctx.enter_context(tc.tile_pool(name="const", bufs=1))
        ones_f = const_pool.tile([P, P], mybir.dt.float32)
        tri_f = const_pool.tile([P, P], mybir.dt.float32)
        ones_r = const_pool.tile([P, P], f32r)
        tri_r = const_pool.tile([P, P], f32r)
        # U = all-ones; T[k, m] = 1 for m >= k (iota = m - k, keep if >= 0).
        # Built in f32 (Memset can't encode an f32r value type), then
        # rounded to f32r by tensor_copy, which the BIR verifier accepts
        # as an fp32r producer.
        nc.gpsimd.memset(ones_f[:, :], 1.0)
        nc.gpsimd.affine_select(
            out=tri_f[:, :],
            in_=ones_f[:, :],
            pattern=[[1, P]],
            compare_op=mybir.AluOpType.is_ge,
            fill=0.0,
            base=0,
            channel_multiplier=-1,
        )
        nc.vector.tensor_copy(out=ones_r[:, :], in_=ones_f[:, :])
        nc.vector.tensor_copy(out=tri_r[:, :], in_=tri_f[:, :])

        in_pool = ctx.enter_context(tc.tile_pool(name="in", bufs=BLOC))
        rnd_pool = ctx.enter_context(tc.tile_pool(name="rnd", bufs=BLOC))
        psum_pool = ctx.enter_context(tc.tile_pool(name="ps", bufs=8, space="PSUM"))

        # All loads up front, back-to-back on SP's queue.
        tin = []
        for b in range(BLOC):
            t = in_pool.tile([P, C, NCH, W], mybir.dt.float32, tag="in")
            nc.sync.dma_start(
                t[:, :, :, :], x[b].rearrange("c (j p) w -> p c j w", p=P)
            )
            tin.append(t)

        for b in range(BLOC):
            t = tin[b]
            o = out_pool.tile([P, C, NCH, W], mybir.dt.float32, tag="out")
            # round channel-1 data to fp32r for the PE (DVE)
            tr = rnd_pool.tile([P, NCH, W], f32r, tag="rnd")
            for j in range(NCH):
                nc.vector.tensor_copy(out=tr[:, j, :], in_=t[:, 1, j, :])
            # channel 1: per-chunk PSUM accumulation group on PE
            for j in range(NCH):
                ps = psum_pool.tile([P, W], mybir.dt.float32, tag="ps")
                for i in range(j):
                    nc.tensor.matmul(
                        out=ps[:, :],
                        lhsT=ones_r[:, :],
                        rhs=tr[:, i, :],
                        start=(i == 0),
                        stop=False,
                    )
                nc.tensor.matmul(
                    out=ps[:, :],
                    lhsT=tri_r[:, :],
                    rhs=tr[:, j, :],
                    start=(j == 0),
                    stop=True,
                )
                nc.scalar.copy(out=o[:, 1, j, :], in_=ps[:, :])
            # channel 0: free-axis prefix scan on DVE
            for j in range(NCH):
                nc.vector.tensor_tensor_scan(
                    out=o[:, 0, j, :],
                    data0=t[:, 0, j, :],
                    data1=t[:, 0, j, :],
                    initial=0.0,
                    op0=add,
                    op1=bypass,
                )
            # stores on gpsimd's SWDGE ring; the final store goes on
            # SP's HWDGE ring (idle once loads are done) so the end-of-kernel
            # drain chain doesn't bottom out on Pool, the barrier coordinator
            eng = nc.sync if b == BLOC - 1 else nc.gpsimd
            eng.dma_start(
                y[b].rearrange("c (j p) w -> p c j w", p=P), o[:, :, :, :]
            )

    nc.compile()
    return nc


def _get_nc():
    global _compiled
    if _compiled is None:
        _compiled = _build()
    return _compiled


def _in_maps(x):
    return [
        {"x": np.ascontiguousarray(x[i * BLOC : (i + 1) * BLOC])}
        for i in range(NCORES)
    ]


def kernel(input_diffgrid):
    from concourse.bass_utils import run_bass_kernel_spmd

    x = np.asarray(input_diffgrid, dtype=np.float32)
    nc = _get_nc()
    res = run_bass_kernel_spmd(nc, _in_maps(x), list(range(NCORES)))
    return np.concatenate(
        [np.asarray(res.results[i]["y"]) for i in range(NCORES)], axis=0
    )
